# revision 3
# baseline (speedup 1.0000x reference)
"""Multi-head attention (B=4, S=2048, D=1024, H=16) on 8 TRN2 NeuronCores.

Sharding: data-parallel over batch (4) x tensor-parallel over head halves (2).
Core c handles batch b = c//2 and heads [8g, 8g+8) where g = c%2.
Each core computes a partial [S, D] output-projection contribution; the host
sums the two head-group partials per batch.

All activations are passed to the device pre-transposed (feature dim on
partitions) so the kernel needs no on-device transposes:
  - projections contract over d (model dim) with host-transposed x^T,
  - scores are built transposed [k, q] so exp() output feeds the P @ V
    matmul directly as the moving operand,
  - P @ [V | 1] yields the softmax denominator as row 64 of U^T for free,
  - normalized U^T tiles are exactly the stationary layout w_o needs.

Matmuls run in float32r (single-pass fp32, ~1e-5 rel err on TRN2).
"""

import numpy as np

B, S, D, H = 4, 2048, 1024, 16
DK = D // H          # 64
G = 2                # head groups (tensor-parallel degree per batch)
HL = H // G          # 8 local heads per core
DV = HL * DK         # 512 local value dim
N_CORES = 8

_cached = {}


def _build():
    import concourse.bass as bass
    import concourse.tile as tile
    from concourse import bacc, mybir

    f32 = mybir.dt.float32
    f32r = mybir.dt.float32r
    EXP = mybir.ActivationFunctionType.Exp

    nc = bacc.Bacc("TRN2", target_bir_lowering=False, debug=False,
                   num_devices=N_CORES)

    xqT = nc.dram_tensor("xqT", [D, S], f32, kind="ExternalInput").ap()
    xkT = nc.dram_tensor("xkT", [D, S], f32, kind="ExternalInput").ap()
    xvT = nc.dram_tensor("xvT", [D, S], f32, kind="ExternalInput").ap()
    wqT = nc.dram_tensor("wqT", [D, DV], f32, kind="ExternalInput").ap()
    wkT = nc.dram_tensor("wkT", [D, DV], f32, kind="ExternalInput").ap()
    wvT = nc.dram_tensor("wvT", [D, DV], f32, kind="ExternalInput").ap()
    woT = nc.dram_tensor("woT", [DV, D], f32, kind="ExternalInput").ap()
    out = nc.dram_tensor("out", [S, D], f32, kind="ExternalOutput").ap()

    ND = D // 128     # 8 d-tiles
    NS = S // 128     # 16 s-tiles (k-tiles)
    NQC = S // 512    # 4 q-chunks
    NT = DV // 128    # 4 dk/dv-tiles

    with tile.TileContext(nc) as tc:
        with tc.tile_pool(name="persist", bufs=1) as persist:
            QT = {}    # [t][c] -> [128, 512] tiles of Q^T (dk rows, q cols)
            KT = {}
            Vaug = {}  # [kt] -> [128, 8, 65]: per-head V columns + ones col

            # ---------------- Phase 1: projections ----------------
            with (
                tc.tile_pool(name="stage", bufs=16) as stage,
                tc.tile_pool(name="wpool", bufs=8) as wpool,
                tc.tile_pool(name="ppj", bufs=4, space=bass.MemorySpace.PSUM) as ppj,
            ):
                # Q^T and K^T: out[dk, q] = sum_d wT[d, dk] * xT[d, q]
                for name, xT, wT, dst in (("q", xqT, wqT, QT), ("k", xkT, wkT, KT)):
                    ws = []
                    for d in range(ND):
                        wt = wpool.tile([128, DV], f32r, tag="w")
                        nc.sync.dma_start(
                            wt[:], wT[128 * d:128 * (d + 1), :].bitcast(f32r))
                        ws.append(wt)
                    for c in range(NQC):
                        xs = []
                        for d in range(ND):
                            xt = stage.tile([128, 512], f32r, tag="act")
                            nc.sync.dma_start(
                                xt[:],
                                xT[128 * d:128 * (d + 1),
                                   512 * c:512 * (c + 1)].bitcast(f32r))
                            xs.append(xt)
                        for t in range(NT):
                            acc = ppj.tile([128, 512], f32)
                            for d in range(ND):
                                nc.tensor.matmul(
                                    acc[:],
                                    ws[d][:, 128 * t:128 * (t + 1)],
                                    xs[d][:],
                                    start=(d == 0), stop=(d == ND - 1))
                            dt_ = persist.tile([128, 512], f32r,
                                               tag=f"{name}T{t}_{c}")
                            nc.vector.tensor_copy(dt_[:], acc[:])
                            dst.setdefault(t, {})[c] = dt_

                # V (natural): out[s, dv] = sum_d xvT[d, s] * wvT[d, dv]
                ws = []
                for d in range(ND):
                    wt = wpool.tile([128, DV], f32r, tag="w")
                    nc.sync.dma_start(
                        wt[:], wvT[128 * d:128 * (d + 1), :].bitcast(f32r))
                    ws.append(wt)
                for c in range(NQC):
                    xs = []
                    for d in range(ND):
                        xt = stage.tile([128, 512], f32r, tag="act")
                        nc.sync.dma_start(
                            xt[:],
                            xvT[128 * d:128 * (d + 1),
                                512 * c:512 * (c + 1)].bitcast(f32r))
                        xs.append(xt)
                    for ktl in range(4):
                        kt = 4 * c + ktl
                        acc = ppj.tile([128, 512], f32)
                        for d in range(ND):
                            nc.tensor.matmul(
                                acc[:],
                                xs[d][:, 128 * ktl:128 * (ktl + 1)],
                                ws[d][:],
                                start=(d == 0), stop=(d == ND - 1))
                        va = persist.tile([128, HL, DK + 1], f32r,
                                          tag=f"vaug{kt}")
                        nc.vector.tensor_copy(
                            va[:, :, 0:DK],
                            acc[:].rearrange("p (h k) -> p h k", h=HL))
                        nc.vector.tensor_copy(
                            va[:, :, DK],
                            nc.const_aps.tensor(1.0, (128, HL), f32))
                        Vaug[kt] = va

            # ---------- Phase 2+3: attention + output projection ----------
            with (
                tc.tile_pool(name="opool", bufs=1) as opool,
                tc.tile_pool(name="ppool", bufs=3) as ppool,
                tc.tile_pool(name="spool", bufs=2, space=bass.MemorySpace.PSUM) as spool,
                tc.tile_pool(name="upool", bufs=2, space=bass.MemorySpace.PSUM) as upool,
                tc.tile_pool(name="rpool", bufs=3) as rpool,
                tc.tile_pool(name="wopool", bufs=1) as wopool,
                tc.tile_pool(name="ppo", bufs=2, space=bass.MemorySpace.PSUM) as ppo,
                tc.tile_pool(name="obuf", bufs=3) as obuf,
            ):
                outT = {}  # [t][qc] -> [128, 512] normalized attention out^T
                wos = []
                for t in range(NT):
                    wo = wopool.tile([128, D], f32r, tag=f"wo{t}")
                    nc.sync.dma_start(
                        wo[:], woT[128 * t:128 * (t + 1), :].bitcast(f32r))
                    wos.append(wo)

                for qc in range(NQC):
                    for h in range(HL):
                        t, po = h // 2, 64 * (h % 2)
                        U = upool.tile([65, 512], f32, tag="u")
                        for kp in range(NS // 2):
                            sc = spool.tile([128, 1024], f32, tag="sc")
                            for j in range(2):
                                kt = 2 * kp + j
                                # scores^T[k, q] = K^T_tile.T @ Q^T chunk
                                nc.tensor.matmul(
                                    sc[:, 512 * j:512 * (j + 1)],
                                    KT[t][kt // 4][po:po + 64,
                                                   128 * (kt % 4):128 * (kt % 4 + 1)],
                                    QT[t][qc][po:po + 64, :],
                                    start=True, stop=True)
                            P = ppool.tile([128, 1024], f32r, tag="p")
                            nc.scalar.activation(P[:], sc[:], EXP, scale=0.125)
                            for j in range(2):
                                kt = 2 * kp + j
                                nc.tensor.matmul(
                                    U[:],
                                    Vaug[kt][:, h, :],
                                    P[:, 512 * j:512 * (j + 1)],
                                    start=(kt == 0), stop=(kt == NS - 1))
                        # normalize rows 0..63 of U by row 64, into out^T.
                        # Engine ops keep all operands on one partition range;
                        # cross-partition moves go through SBUF-SBUF DMA.
                        r64 = rpool.tile([65, 512], f32, tag="r64")
                        nc.vector.tensor_copy(r64[64:65, :], U[64:65, :])
                        rrow = rpool.tile([1, 512], f32, tag="rrow")
                        nc.sync.dma_start(rrow[:], r64[64:65, :])
                        rrec = rpool.tile([1, 512], f32, tag="rrec")
                        nc.vector.reciprocal_approx_fast(rrec[:], rrow[:])
                        rb = rpool.tile([64, 512], f32, tag="rb")
                        nc.gpsimd.partition_broadcast(rb[:], rrec[:])
                        ot = outT.setdefault(t, {}).get(qc)
                        if ot is None:
                            ot = opool.tile([128, 512], f32r, tag=f"oT{t}_{qc}")
                            outT[t][qc] = ot
                        if po == 0:
                            nc.vector.tensor_mul(ot[0:64, :], U[0:64, :], rb[:])
                        else:
                            stg = rpool.tile([64, 512], f32r, tag="stg")
                            nc.vector.tensor_mul(stg[:], U[0:64, :], rb[:])
                            nc.sync.dma_start(ot[64:128, :], stg[:])

                    # output projection for this q-chunk:
                    # final[s, n] = sum_dv outT[dv, s] * woT[dv, n]
                    for st in range(4):
                        for ncol in range(2):
                            acc = ppo.tile([128, 512], f32)
                            for t in range(NT):
                                nc.tensor.matmul(
                                    acc[:],
                                    outT[t][qc][:, 128 * st:128 * (st + 1)],
                                    wos[t][:, 512 * ncol:512 * (ncol + 1)],
                                    start=(t == 0), stop=(t == NT - 1))
                            ob = obuf.tile([128, 512], f32, tag="ob")
                            nc.vector.tensor_copy(ob[:], acc[:])
                            nc.sync.dma_start(
                                out[512 * qc + 128 * st:512 * qc + 128 * (st + 1),
                                    512 * ncol:512 * (ncol + 1)],
                                ob[:])

    nc.compile()
    return nc


def kernel(query, key, value, w_q, w_k, w_v, w_o):
    from concourse.bass_utils import run_bass_kernel_spmd

    if "nc" not in _cached:
        _cached["nc"] = _build()
    nc = _cached["nc"]

    query = np.asarray(query, dtype=np.float32)
    key = np.asarray(key, dtype=np.float32)
    value = np.asarray(value, dtype=np.float32)
    w_q = np.asarray(w_q, dtype=np.float32)
    w_k = np.asarray(w_k, dtype=np.float32)
    w_v = np.asarray(w_v, dtype=np.float32)
    w_o = np.asarray(w_o, dtype=np.float32)

    c = np.ascontiguousarray
    in_maps = []
    for core in range(N_CORES):
        b, g = core // G, core % G
        rows = slice(DV * g, DV * (g + 1))
        in_maps.append({
            "xqT": c(query[b].T),
            "xkT": c(key[b].T),
            "xvT": c(value[b].T),
            "wqT": c(w_q[rows, :].T),
            "wkT": c(w_k[rows, :].T),
            "wvT": c(w_v[rows, :].T),
            "woT": c(w_o[:, rows].T),
        })

    res = run_bass_kernel_spmd(nc, in_maps, list(range(N_CORES)))
    full = np.empty((B, S, D), np.float32)
    for b in range(B):
        full[b] = res.results[G * b]["out"] + res.results[G * b + 1]["out"]
    return full


# revision 6
# speedup vs baseline: 1.0971x; 1.0971x over previous
"""Multi-head attention (B=4, S=2048, D=1024, H=16) on 8 TRN2 NeuronCores.

Sharding: data-parallel over batch (4) x tensor-parallel over head halves (2).
Core c handles batch b = c//2 and heads [8g, 8g+8) where g = c%2.
Each core computes a partial [S, D] output-projection contribution; the host
sums the two head-group partials per batch.

All activations are passed to the device pre-transposed (feature dim on
partitions) so the kernel needs no on-device transposes:
  - projections contract over d (model dim) with host-transposed x^T,
  - scores are built transposed [k, q] so exp() output feeds the P @ V
    matmul directly as the moving operand,
  - P @ [V | 1] yields the softmax denominator as row 64 of U^T for free,
  - normalized U^T tiles are exactly the stationary layout w_o needs.

Matmul operands are bf16 (fp32 PSUM accumulation); fp32 moving operands
stream at half rate on TRN2, bf16 at full rate. Head pairs share the PE
array via row tiling (partitions 0-63 / 64-127) so the DK=64 score matmuls
run concurrently.
"""

import numpy as np

B, S, D, H = 4, 2048, 1024, 16
DK = D // H          # 64
G = 2                # head groups (tensor-parallel degree per batch)
HL = H // G          # 8 local heads per core
DV = HL * DK         # 512 local value dim
N_CORES = 8

_cached = {}


def _build():
    import concourse.bass as bass
    import concourse.tile as tile
    from concourse import bacc, mybir

    f32 = mybir.dt.float32
    bf16 = mybir.dt.bfloat16
    EXP = mybir.ActivationFunctionType.Exp

    nc = bacc.Bacc("TRN2", target_bir_lowering=False, debug=False,
                   num_devices=N_CORES)

    xqT = nc.dram_tensor("xqT", [D, S], bf16, kind="ExternalInput").ap()
    xkT = nc.dram_tensor("xkT", [D, S], bf16, kind="ExternalInput").ap()
    xvT = nc.dram_tensor("xvT", [D, S], bf16, kind="ExternalInput").ap()
    wqT = nc.dram_tensor("wqT", [D, DV], bf16, kind="ExternalInput").ap()
    wkT = nc.dram_tensor("wkT", [D, DV], bf16, kind="ExternalInput").ap()
    wvT = nc.dram_tensor("wvT", [D, DV], bf16, kind="ExternalInput").ap()
    woT = nc.dram_tensor("woT", [DV, D], bf16, kind="ExternalInput").ap()
    out = nc.dram_tensor("out", [S, D], f32, kind="ExternalOutput").ap()

    ND = D // 128     # 8 d-tiles
    NS = S // 128     # 16 s-tiles (k-tiles)
    NQC = S // 512    # 4 q-chunks
    NT = DV // 128    # 4 dk/dv-tiles

    with tile.TileContext(nc) as tc:
        with tc.tile_pool(name="persist", bufs=1) as persist:
            QT = {}    # [t][c] -> [128, 512] tiles of Q^T (dk rows, q cols)
            KT = {}
            Vaug = {}  # [kt] -> [128, 8, 65]: per-head V columns + ones col

            # ---------------- Phase 1: projections ----------------
            with (
                tc.tile_pool(name="stage", bufs=16) as stage,
                tc.tile_pool(name="wpool", bufs=8) as wpool,
                tc.tile_pool(name="ppj", bufs=4, space=bass.MemorySpace.PSUM) as ppj,
            ):
                # Q^T and K^T: out[dk, q] = sum_d wT[d, dk] * xT[d, q]
                for name, xT, wT, dst in (("q", xqT, wqT, QT), ("k", xkT, wkT, KT)):
                    ws = []
                    for d in range(ND):
                        wt = wpool.tile([128, DV], bf16, tag="w")
                        nc.sync.dma_start(wt[:], wT[128 * d:128 * (d + 1), :])
                        ws.append(wt)
                    for c in range(NQC):
                        xs = []
                        for d in range(ND):
                            xt = stage.tile([128, 512], bf16, tag="act")
                            nc.sync.dma_start(
                                xt[:],
                                xT[128 * d:128 * (d + 1), 512 * c:512 * (c + 1)])
                            xs.append(xt)
                        for t in range(NT):
                            acc = ppj.tile([128, 512], f32)
                            for d in range(ND):
                                nc.tensor.matmul(
                                    acc[:],
                                    ws[d][:, 128 * t:128 * (t + 1)],
                                    xs[d][:],
                                    start=(d == 0), stop=(d == ND - 1))
                            dt_ = persist.tile([128, 512], bf16,
                                               tag=f"{name}T{t}_{c}")
                            nc.vector.tensor_copy(dt_[:], acc[:])
                            dst.setdefault(t, {})[c] = dt_

                # V (natural): out[s, dv] = sum_d xvT[d, s] * wvT[d, dv]
                ws = []
                for d in range(ND):
                    wt = wpool.tile([128, DV], bf16, tag="w")
                    nc.sync.dma_start(wt[:], wvT[128 * d:128 * (d + 1), :])
                    ws.append(wt)
                for c in range(NQC):
                    xs = []
                    for d in range(ND):
                        xt = stage.tile([128, 512], bf16, tag="act")
                        nc.sync.dma_start(
                            xt[:],
                            xvT[128 * d:128 * (d + 1), 512 * c:512 * (c + 1)])
                        xs.append(xt)
                    for ktl in range(4):
                        kt = 4 * c + ktl
                        acc = ppj.tile([128, 512], f32)
                        for d in range(ND):
                            nc.tensor.matmul(
                                acc[:],
                                xs[d][:, 128 * ktl:128 * (ktl + 1)],
                                ws[d][:],
                                start=(d == 0), stop=(d == ND - 1))
                        va = persist.tile([128, HL, DK + 1], bf16,
                                          tag=f"vaug{kt}")
                        nc.vector.tensor_copy(
                            va[:, :, 0:DK],
                            acc[:].rearrange("p (h k) -> p h k", h=HL))
                        nc.vector.tensor_copy(
                            va[:, :, DK],
                            nc.const_aps.tensor(1.0, (128, HL), bf16))
                        Vaug[kt] = va

            # ---------- Phase 2+3: attention + output projection ----------
            with (
                tc.tile_pool(name="opool", bufs=1) as opool,
                tc.tile_pool(name="ppool", bufs=4) as ppool,
                tc.tile_pool(name="spool", bufs=2, space=bass.MemorySpace.PSUM) as spool,
                tc.tile_pool(name="upool", bufs=2, space=bass.MemorySpace.PSUM) as upool,
                tc.tile_pool(name="rpool", bufs=3) as rpool,
                tc.tile_pool(name="wopool", bufs=1) as wopool,
                tc.tile_pool(name="ppo", bufs=2, space=bass.MemorySpace.PSUM) as ppo,
                tc.tile_pool(name="obuf", bufs=3) as obuf,
            ):
                outT = {}  # [t][qc] -> [128, 512] normalized attention out^T
                wos = []
                for t in range(NT):
                    wo = wopool.tile([128, D], bf16, tag=f"wo{t}")
                    nc.sync.dma_start(wo[:], woT[128 * t:128 * (t + 1), :])
                    wos.append(wo)

                for qc in range(NQC):
                    for hp in range(HL // 2):
                        # head pair (2hp, 2hp+1) = partition halves of tile hp:
                        # their DK=64 score matmuls row-tile the PE array.
                        t = hp
                        U = [upool.tile([65, 512], f32, tag="u", name="u")
                             for _ in range(2)]
                        for kp in range(NS // 2):
                            sc = [spool.tile([128, 1024], f32, tag="sc",
                                             name="sc") for _ in range(2)]
                            for j in range(2):
                                kt = 2 * kp + j
                                for i in range(2):
                                    po = 64 * i
                                    nc.tensor.matmul(
                                        sc[i][:, 512 * j:512 * (j + 1)],
                                        KT[t][kt // 4][po:po + 64,
                                                       128 * (kt % 4):128 * (kt % 4 + 1)],
                                        QT[t][qc][po:po + 64, :],
                                        start=True, stop=True)
                            for i in range(2):
                                P = ppool.tile([128, 1024], bf16, tag="p")
                                nc.scalar.activation(P[:], sc[i][:], EXP,
                                                     scale=0.125)
                                for j in range(2):
                                    kt = 2 * kp + j
                                    nc.tensor.matmul(
                                        U[i][:],
                                        Vaug[kt][:, 2 * hp + i, :],
                                        P[:, 512 * j:512 * (j + 1)],
                                        start=(kt == 0), stop=(kt == NS - 1))
                        # normalize rows 0..63 of U by row 64, into out^T.
                        # Engine ops keep all operands on one partition range;
                        # cross-partition moves go through SBUF-SBUF DMA.
                        ot = outT.setdefault(t, {}).get(qc)
                        if ot is None:
                            ot = opool.tile([128, 512], bf16, tag=f"oT{t}_{qc}")
                            outT[t][qc] = ot
                        for i in range(2):
                            r64 = rpool.tile([65, 512], f32, tag="r64")
                            nc.vector.tensor_copy(r64[64:65, :], U[i][64:65, :])
                            rrow = rpool.tile([1, 512], f32, tag="rrow")
                            nc.sync.dma_start(rrow[:], r64[64:65, :])
                            rrec = rpool.tile([1, 512], f32, tag="rrec")
                            nc.vector.reciprocal_approx_fast(rrec[:], rrow[:])
                            rb = rpool.tile([64, 512], f32, tag="rb")
                            nc.gpsimd.partition_broadcast(rb[:], rrec[:])
                            if i == 0:
                                nc.vector.tensor_mul(ot[0:64, :], U[i][0:64, :],
                                                     rb[:])
                            else:
                                stg = rpool.tile([64, 512], bf16, tag="stg")
                                nc.vector.tensor_mul(stg[:], U[i][0:64, :], rb[:])
                                nc.sync.dma_start(ot[64:128, :], stg[:])

                    # output projection for this q-chunk:
                    # final[s, n] = sum_dv outT[dv, s] * woT[dv, n]
                    for st in range(4):
                        for ncol in range(2):
                            acc = ppo.tile([128, 512], f32)
                            for t in range(NT):
                                nc.tensor.matmul(
                                    acc[:],
                                    outT[t][qc][:, 128 * st:128 * (st + 1)],
                                    wos[t][:, 512 * ncol:512 * (ncol + 1)],
                                    start=(t == 0), stop=(t == NT - 1))
                            ob = obuf.tile([128, 512], f32, tag="ob")
                            nc.vector.tensor_copy(ob[:], acc[:])
                            nc.sync.dma_start(
                                out[512 * qc + 128 * st:512 * qc + 128 * (st + 1),
                                    512 * ncol:512 * (ncol + 1)],
                                ob[:])

    nc.compile()
    return nc


def kernel(query, key, value, w_q, w_k, w_v, w_o):
    import ml_dtypes
    from concourse.bass_utils import run_bass_kernel_spmd

    if "nc" not in _cached:
        _cached["nc"] = _build()
    nc = _cached["nc"]

    bf = ml_dtypes.bfloat16
    query = np.asarray(query, dtype=np.float32)
    key = np.asarray(key, dtype=np.float32)
    value = np.asarray(value, dtype=np.float32)
    w_q = np.asarray(w_q, dtype=np.float32)
    w_k = np.asarray(w_k, dtype=np.float32)
    w_v = np.asarray(w_v, dtype=np.float32)
    w_o = np.asarray(w_o, dtype=np.float32)

    def c(a):
        return np.ascontiguousarray(a).astype(bf)

    in_maps = []
    for core in range(N_CORES):
        b, g = core // G, core % G
        rows = slice(DV * g, DV * (g + 1))
        in_maps.append({
            "xqT": c(query[b].T),
            "xkT": c(key[b].T),
            "xvT": c(value[b].T),
            "wqT": c(w_q[rows, :].T),
            "wkT": c(w_k[rows, :].T),
            "wvT": c(w_v[rows, :].T),
            "woT": c(w_o[:, rows].T),
        })

    res = run_bass_kernel_spmd(nc, in_maps, list(range(N_CORES)))
    full = np.empty((B, S, D), np.float32)
    for b in range(B):
        full[b] = res.results[G * b]["out"] + res.results[G * b + 1]["out"]
    return full


# revision 7
# speedup vs baseline: 1.1836x; 1.0788x over previous
"""Multi-head attention (B=4, S=2048, D=1024, H=16) on 8 TRN2 NeuronCores.

Sharding: data-parallel over batch (4) x tensor-parallel over head halves (2).
Core c handles batch b = c//2 and heads [8g, 8g+8) where g = c%2.
Each core computes a partial [S, D] output-projection contribution; the host
sums the two head-group partials per batch.

All activations are passed to the device pre-transposed (feature dim on
partitions) so the kernel needs no on-device transposes:
  - projections contract over d (model dim) with host-transposed x^T,
  - scores are built transposed [k, q] so exp() output feeds the P @ V
    matmul directly as the moving operand,
  - P @ [V | 1] yields the softmax denominator as row 64 of U^T for free,
  - normalized U^T tiles are exactly the stationary layout w_o needs.

Matmul operands are bf16 (fp32 PSUM accumulation); fp32 moving operands
stream at half rate on TRN2, bf16 at full rate. Head pairs share the PE
array via row tiling (partitions 0-63 / 64-127) so the DK=64 score matmuls
run concurrently. K/V projections run first, then each q-chunk's Q
projection immediately precedes its attention so the scalar engine starts
exp() work as early as possible.

PSUM budget (8 banks): 3 x [128,1024] score tiles (6 banks) + one shared
2-slot pool for every [<=128,512] accumulator (projection accs, U tiles,
w_o accs).
"""

import numpy as np

B, S, D, H = 4, 2048, 1024, 16
DK = D // H          # 64
G = 2                # head groups (tensor-parallel degree per batch)
HL = H // G          # 8 local heads per core
DV = HL * DK         # 512 local value dim
N_CORES = 8

_cached = {}


def _build():
    import concourse.bass as bass
    import concourse.tile as tile
    from concourse import bacc, mybir

    f32 = mybir.dt.float32
    bf16 = mybir.dt.bfloat16
    EXP = mybir.ActivationFunctionType.Exp

    nc = bacc.Bacc("TRN2", target_bir_lowering=False, debug=False,
                   num_devices=N_CORES)

    xqT = nc.dram_tensor("xqT", [D, S], bf16, kind="ExternalInput").ap()
    xkT = nc.dram_tensor("xkT", [D, S], bf16, kind="ExternalInput").ap()
    xvT = nc.dram_tensor("xvT", [D, S], bf16, kind="ExternalInput").ap()
    wqT = nc.dram_tensor("wqT", [D, DV], bf16, kind="ExternalInput").ap()
    wkT = nc.dram_tensor("wkT", [D, DV], bf16, kind="ExternalInput").ap()
    wvT = nc.dram_tensor("wvT", [D, DV], bf16, kind="ExternalInput").ap()
    woT = nc.dram_tensor("woT", [DV, D], bf16, kind="ExternalInput").ap()
    out = nc.dram_tensor("out", [S, D], f32, kind="ExternalOutput").ap()

    ND = D // 128     # 8 d-tiles
    NS = S // 128     # 16 s-tiles (k-tiles)
    NQC = S // 512    # 4 q-chunks
    NT = DV // 128    # 4 dk/dv-tiles

    with tile.TileContext(nc) as tc:
        with (
            tc.tile_pool(name="persist", bufs=1) as persist,
            tc.tile_pool(name="stage", bufs=16) as stage,
            tc.tile_pool(name="wpool", bufs=8) as wpool,
            tc.tile_pool(name="spool", bufs=3, space=bass.MemorySpace.PSUM) as spool,
            tc.tile_pool(name="upool", bufs=2, space=bass.MemorySpace.PSUM) as upool,
            tc.tile_pool(name="ppool", bufs=4) as ppool,
            tc.tile_pool(name="rpool", bufs=3) as rpool,
            tc.tile_pool(name="obuf", bufs=3) as obuf,
        ):
            QT = {}    # [t][c] -> [128, 512] tiles of Q^T (dk rows, q cols)
            KT = {}
            Vaug = {}  # [kt] -> [128, 8, 65]: per-head V columns + ones col

            def uacc(shape):
                return upool.tile(shape, f32, tag="u", name="uacc")

            # ---------------- K / V projections ----------------
            # K^T: out[dk, k] = sum_d wkT[d, dk] * xkT[d, k]
            ws = []
            for d in range(ND):
                wt = wpool.tile([128, DV], bf16, tag="w", name="wk")
                nc.sync.dma_start(wt[:], wkT[128 * d:128 * (d + 1), :])
                ws.append(wt)
            for c in range(NQC):
                xs = []
                for d in range(ND):
                    xt = stage.tile([128, 512], bf16, tag="act", name="xk")
                    nc.sync.dma_start(
                        xt[:], xkT[128 * d:128 * (d + 1), 512 * c:512 * (c + 1)])
                    xs.append(xt)
                for t in range(NT):
                    acc = uacc([128, 512])
                    for d in range(ND):
                        nc.tensor.matmul(
                            acc[:], ws[d][:, 128 * t:128 * (t + 1)], xs[d][:],
                            start=(d == 0), stop=(d == ND - 1))
                    dt_ = persist.tile([128, 512], bf16, tag=f"kT{t}_{c}",
                                       name="kT")
                    nc.vector.tensor_copy(dt_[:], acc[:])
                    KT.setdefault(t, {})[c] = dt_

            # V (natural): out[s, dv] = sum_d xvT[d, s] * wvT[d, dv]
            ws = []
            for d in range(ND):
                wt = wpool.tile([128, DV], bf16, tag="w", name="wv")
                nc.sync.dma_start(wt[:], wvT[128 * d:128 * (d + 1), :])
                ws.append(wt)
            for c in range(NQC):
                xs = []
                for d in range(ND):
                    xt = stage.tile([128, 512], bf16, tag="act", name="xv")
                    nc.sync.dma_start(
                        xt[:], xvT[128 * d:128 * (d + 1), 512 * c:512 * (c + 1)])
                    xs.append(xt)
                for ktl in range(4):
                    kt = 4 * c + ktl
                    acc = uacc([128, 512])
                    for d in range(ND):
                        nc.tensor.matmul(
                            acc[:], xs[d][:, 128 * ktl:128 * (ktl + 1)], ws[d][:],
                            start=(d == 0), stop=(d == ND - 1))
                    va = persist.tile([128, HL, DK + 1], bf16, tag=f"vaug{kt}",
                                      name="vaug")
                    nc.vector.tensor_copy(
                        va[:, :, 0:DK],
                        acc[:].rearrange("p (h k) -> p h k", h=HL))
                    nc.vector.tensor_copy(
                        va[:, :, DK], nc.const_aps.tensor(1.0, (128, HL), bf16))
                    Vaug[kt] = va

            # Q projection weights + w_o weights (loaded once)
            wqs = []
            for d in range(ND):
                wt = wpool.tile([128, DV], bf16, tag="w", name="wq")
                nc.sync.dma_start(wt[:], wqT[128 * d:128 * (d + 1), :])
                wqs.append(wt)
            wos = []
            for t in range(NT):
                wo = wpool.tile([128, D], bf16, tag=f"wo{t}", name="wo")
                nc.sync.dma_start(wo[:], woT[128 * t:128 * (t + 1), :])
                wos.append(wo)

            # -------- per q-chunk: Q projection, attention, out-projection ----
            for qc in range(NQC):
                xs = []
                for d in range(ND):
                    xt = stage.tile([128, 512], bf16, tag="act", name="xq")
                    nc.sync.dma_start(
                        xt[:], xqT[128 * d:128 * (d + 1), 512 * qc:512 * (qc + 1)])
                    xs.append(xt)
                for t in range(NT):
                    acc = uacc([128, 512])
                    for d in range(ND):
                        nc.tensor.matmul(
                            acc[:], wqs[d][:, 128 * t:128 * (t + 1)], xs[d][:],
                            start=(d == 0), stop=(d == ND - 1))
                    dt_ = persist.tile([128, 512], bf16, tag=f"qT{t}_{qc}",
                                       name="qT")
                    nc.vector.tensor_copy(dt_[:], acc[:])
                    QT.setdefault(t, {})[qc] = dt_

                outT = {}
                for hp in range(HL // 2):
                    # head pair (2hp, 2hp+1) = partition halves of tile hp:
                    # their DK=64 score matmuls row-tile the PE array.
                    t = hp
                    U = [uacc([65, 512]) for _ in range(2)]
                    for kp in range(NS // 2):
                        sc = [spool.tile([128, 1024], f32, tag="sc",
                                         name="sc") for _ in range(2)]
                        for j in range(2):
                            kt = 2 * kp + j
                            for i in range(2):
                                po = 64 * i
                                nc.tensor.matmul(
                                    sc[i][:, 512 * j:512 * (j + 1)],
                                    KT[t][kt // 4][po:po + 64,
                                                   128 * (kt % 4):128 * (kt % 4 + 1)],
                                    QT[t][qc][po:po + 64, :],
                                    start=True, stop=True)
                        for i in range(2):
                            P = ppool.tile([128, 1024], bf16, tag="p", name="p")
                            nc.scalar.activation(P[:], sc[i][:], EXP, scale=0.125)
                            for j in range(2):
                                kt = 2 * kp + j
                                nc.tensor.matmul(
                                    U[i][:],
                                    Vaug[kt][:, 2 * hp + i, :],
                                    P[:, 512 * j:512 * (j + 1)],
                                    start=(kt == 0), stop=(kt == NS - 1))
                    # normalize rows 0..63 of U by row 64, into out^T.
                    # Engine ops keep all operands on one partition range;
                    # cross-partition moves go through SBUF-SBUF DMA.
                    ot = persist.tile([128, 512], bf16, tag=f"oT{t}_{qc % 2}",
                                      name="oT")
                    outT[t] = ot
                    for i in range(2):
                        r64 = rpool.tile([65, 512], f32, tag="r64", name="r64")
                        nc.vector.tensor_copy(r64[64:65, :], U[i][64:65, :])
                        rrow = rpool.tile([1, 512], f32, tag="rrow", name="rrow")
                        nc.sync.dma_start(rrow[:], r64[64:65, :])
                        rrec = rpool.tile([1, 512], f32, tag="rrec", name="rrec")
                        nc.vector.reciprocal_approx_fast(rrec[:], rrow[:])
                        rb = rpool.tile([64, 512], f32, tag="rb", name="rb")
                        nc.gpsimd.partition_broadcast(rb[:], rrec[:])
                        if i == 0:
                            nc.vector.tensor_mul(ot[0:64, :], U[i][0:64, :], rb[:])
                        else:
                            stg = rpool.tile([64, 512], bf16, tag="stg",
                                             name="stg")
                            nc.vector.tensor_mul(stg[:], U[i][0:64, :], rb[:])
                            nc.sync.dma_start(ot[64:128, :], stg[:])

                # output projection for this q-chunk:
                # final[s, n] = sum_dv outT[dv, s] * woT[dv, n]
                for st in range(4):
                    for ncol in range(2):
                        acc = uacc([128, 512])
                        for t in range(NT):
                            nc.tensor.matmul(
                                acc[:],
                                outT[t][:, 128 * st:128 * (st + 1)],
                                wos[t][:, 512 * ncol:512 * (ncol + 1)],
                                start=(t == 0), stop=(t == NT - 1))
                        ob = obuf.tile([128, 512], f32, tag="ob", name="ob")
                        nc.vector.tensor_copy(ob[:], acc[:])
                        nc.sync.dma_start(
                            out[512 * qc + 128 * st:512 * qc + 128 * (st + 1),
                                512 * ncol:512 * (ncol + 1)],
                            ob[:])

    nc.compile()
    return nc


def kernel(query, key, value, w_q, w_k, w_v, w_o):
    import ml_dtypes
    from concourse.bass_utils import run_bass_kernel_spmd

    if "nc" not in _cached:
        _cached["nc"] = _build()
    nc = _cached["nc"]

    bf = ml_dtypes.bfloat16
    query = np.asarray(query, dtype=np.float32)
    key = np.asarray(key, dtype=np.float32)
    value = np.asarray(value, dtype=np.float32)
    w_q = np.asarray(w_q, dtype=np.float32)
    w_k = np.asarray(w_k, dtype=np.float32)
    w_v = np.asarray(w_v, dtype=np.float32)
    w_o = np.asarray(w_o, dtype=np.float32)

    def c(a):
        return np.ascontiguousarray(a).astype(bf)

    in_maps = []
    for core in range(N_CORES):
        b, g = core // G, core % G
        rows = slice(DV * g, DV * (g + 1))
        in_maps.append({
            "xqT": c(query[b].T),
            "xkT": c(key[b].T),
            "xvT": c(value[b].T),
            "wqT": c(w_q[rows, :].T),
            "wkT": c(w_k[rows, :].T),
            "wvT": c(w_v[rows, :].T),
            "woT": c(w_o[:, rows].T),
        })

    res = run_bass_kernel_spmd(nc, in_maps, list(range(N_CORES)))
    full = np.empty((B, S, D), np.float32)
    for b in range(B):
        full[b] = res.results[G * b]["out"] + res.results[G * b + 1]["out"]
    return full


# revision 8
# speedup vs baseline: 1.2380x; 1.0460x over previous
"""Multi-head attention (B=4, S=2048, D=1024, H=16) on 8 TRN2 NeuronCores.

Sharding: data-parallel over batch (4) x tensor-parallel over head halves (2).
Core c handles batch b = c//2 and heads [8g, 8g+8) where g = c%2.
Each core computes a partial [S, D] output-projection contribution; the host
sums the two head-group partials per batch.

All activations are passed to the device pre-transposed (feature dim on
partitions) so the kernel needs no on-device transposes:
  - projections contract over d (model dim) with host-transposed x^T,
  - scores are built transposed [k, q] so exp() output feeds the P @ V
    matmul directly as the moving operand,
  - P @ [V | 1] yields the softmax denominator as row 64 of U^T for free,
  - normalized U^T tiles are exactly the stationary layout w_o needs.

Matmul operands are bf16 (fp32 PSUM accumulation); fp32 moving operands
stream at half rate on TRN2, bf16 at full rate. Head pairs share the PE
array via row tiling (partitions 0-63 / 64-127) so the DK=64 score matmuls
run concurrently. K/V projections run first, then each q-chunk's Q
projection immediately precedes its attention so the scalar engine starts
exp() work as early as possible.

PSUM budget (8 banks): 3 x [128,1024] score tiles (6 banks) + one shared
2-slot pool for every [<=128,512] accumulator (projection accs, U tiles,
w_o accs).
"""

import numpy as np

B, S, D, H = 4, 2048, 1024, 16
DK = D // H          # 64
G = 2                # head groups (tensor-parallel degree per batch)
HL = H // G          # 8 local heads per core
DV = HL * DK         # 512 local value dim
N_CORES = 8

_cached = {}


def _build():
    import concourse.bass as bass
    import concourse.tile as tile
    from concourse import bacc, mybir

    f32 = mybir.dt.float32
    bf16 = mybir.dt.bfloat16
    EXP = mybir.ActivationFunctionType.Exp

    nc = bacc.Bacc("TRN2", target_bir_lowering=False, debug=False,
                   num_devices=N_CORES)

    xqT = nc.dram_tensor("xqT", [D, S], bf16, kind="ExternalInput").ap()
    xkT = nc.dram_tensor("xkT", [D, S], bf16, kind="ExternalInput").ap()
    xvT = nc.dram_tensor("xvT", [D, S], bf16, kind="ExternalInput").ap()
    wqT = nc.dram_tensor("wqT", [D, DV], bf16, kind="ExternalInput").ap()
    wkT = nc.dram_tensor("wkT", [D, DV], bf16, kind="ExternalInput").ap()
    wvT = nc.dram_tensor("wvT", [D, DV], bf16, kind="ExternalInput").ap()
    woT = nc.dram_tensor("woT", [DV, D], bf16, kind="ExternalInput").ap()
    out = nc.dram_tensor("out", [S, D], f32, kind="ExternalOutput").ap()

    ND = D // 128     # 8 d-tiles
    NS = S // 128     # 16 s-tiles (k-tiles)
    NQC = S // 512    # 4 q-chunks
    NT = DV // 128    # 4 dk/dv-tiles

    with tile.TileContext(nc) as tc:
        with (
            tc.tile_pool(name="persist", bufs=1) as persist,
            tc.tile_pool(name="stage", bufs=16) as stage,
            tc.tile_pool(name="wpool", bufs=8) as wpool,
            tc.tile_pool(name="spool", bufs=3, space=bass.MemorySpace.PSUM) as spool,
            tc.tile_pool(name="upool", bufs=2, space=bass.MemorySpace.PSUM) as upool,
            tc.tile_pool(name="ppool", bufs=4) as ppool,
            tc.tile_pool(name="rpool", bufs=3) as rpool,
            tc.tile_pool(name="obuf", bufs=3) as obuf,
        ):
            QT = {}    # [t][c] -> [128, 512] tiles of Q^T (dk rows, q cols)
            KT = {}
            Vaug = {}  # [kt] -> [128, 8, 65]: per-head V columns + ones col

            def uacc(shape):
                return upool.tile(shape, f32, tag="u", name="uacc")

            # ---------------- K / V projections ----------------
            # K^T: out[dk, k] = sum_d wkT[d, dk] * xkT[d, k]
            ws = []
            for d in range(ND):
                wt = wpool.tile([128, DV], bf16, tag="w", name="wk")
                nc.sync.dma_start(wt[:], wkT[128 * d:128 * (d + 1), :])
                ws.append(wt)
            for c in range(NQC):
                xs = []
                for d in range(ND):
                    xt = stage.tile([128, 512], bf16, tag="act", name="xk")
                    nc.sync.dma_start(
                        xt[:], xkT[128 * d:128 * (d + 1), 512 * c:512 * (c + 1)])
                    xs.append(xt)
                for t in range(NT):
                    acc = uacc([128, 512])
                    for d in range(ND):
                        nc.tensor.matmul(
                            acc[:], ws[d][:, 128 * t:128 * (t + 1)], xs[d][:],
                            start=(d == 0), stop=(d == ND - 1))
                    dt_ = persist.tile([128, 512], bf16, tag=f"kT{t}_{c}",
                                       name="kT")
                    nc.vector.tensor_copy(dt_[:], acc[:])
                    KT.setdefault(t, {})[c] = dt_

            # V (natural): out[s, dv] = sum_d xvT[d, s] * wvT[d, dv]
            ws = []
            for d in range(ND):
                wt = wpool.tile([128, DV], bf16, tag="w", name="wv")
                nc.sync.dma_start(wt[:], wvT[128 * d:128 * (d + 1), :])
                ws.append(wt)
            for c in range(NQC):
                xs = []
                for d in range(ND):
                    xt = stage.tile([128, 512], bf16, tag="act", name="xv")
                    nc.sync.dma_start(
                        xt[:], xvT[128 * d:128 * (d + 1), 512 * c:512 * (c + 1)])
                    xs.append(xt)
                for ktl in range(4):
                    kt = 4 * c + ktl
                    acc = uacc([128, 512])
                    for d in range(ND):
                        nc.tensor.matmul(
                            acc[:], xs[d][:, 128 * ktl:128 * (ktl + 1)], ws[d][:],
                            start=(d == 0), stop=(d == ND - 1))
                    va = persist.tile([128, HL, DK + 1], bf16, tag=f"vaug{kt}",
                                      name="vaug")
                    nc.vector.tensor_copy(
                        va[:, :, 0:DK],
                        acc[:].rearrange("p (h k) -> p h k", h=HL))
                    nc.vector.tensor_copy(
                        va[:, :, DK], nc.const_aps.tensor(1.0, (128, HL), bf16))
                    Vaug[kt] = va

            # Q projection weights + w_o weights (loaded once)
            wqs = []
            for d in range(ND):
                wt = wpool.tile([128, DV], bf16, tag="w", name="wq")
                nc.sync.dma_start(wt[:], wqT[128 * d:128 * (d + 1), :])
                wqs.append(wt)
            wos = []
            for t in range(NT):
                wo = wpool.tile([128, D], bf16, tag=f"wo{t}", name="wo")
                nc.sync.dma_start(wo[:], woT[128 * t:128 * (t + 1), :])
                wos.append(wo)

            # -------- per q-chunk: Q projection, attention, out-projection ----
            for qc in range(NQC):
                xs = []
                for d in range(ND):
                    xt = stage.tile([128, 512], bf16, tag="act", name="xq")
                    nc.sync.dma_start(
                        xt[:], xqT[128 * d:128 * (d + 1), 512 * qc:512 * (qc + 1)])
                    xs.append(xt)
                for t in range(NT):
                    acc = uacc([128, 512])
                    for d in range(ND):
                        nc.tensor.matmul(
                            acc[:], wqs[d][:, 128 * t:128 * (t + 1)], xs[d][:],
                            start=(d == 0), stop=(d == ND - 1))
                    dt_ = persist.tile([128, 512], bf16, tag=f"qT{t}_{qc}",
                                       name="qT")
                    nc.vector.tensor_copy(dt_[:], acc[:])
                    QT.setdefault(t, {})[qc] = dt_

                outT = {}
                for hp in range(HL // 2):
                    # head pair (2hp, 2hp+1) = partition halves of tile hp:
                    # their DK=64 score matmuls row-tile the PE array.
                    # U accumulates in SBUF (DVE adds of 4-kt PSUM partials)
                    # so no PSUM slot is held across the whole pair.
                    t = hp
                    Usb = [rpool.tile([65, 512], f32, tag=f"usb{i}", name="usb")
                           for i in range(2)]
                    Up = [None, None]
                    for kp in range(NS // 2):
                        sc = [spool.tile([128, 1024], f32, tag="sc",
                                         name="sc") for _ in range(2)]
                        for j in range(2):
                            kt = 2 * kp + j
                            for i in range(2):
                                po = 64 * i
                                nc.tensor.matmul(
                                    sc[i][:, 512 * j:512 * (j + 1)],
                                    KT[t][kt // 4][po:po + 64,
                                                   128 * (kt % 4):128 * (kt % 4 + 1)],
                                    QT[t][qc][po:po + 64, :],
                                    start=True, stop=True)
                        for i in range(2):
                            P = ppool.tile([128, 1024], bf16, tag="p", name="p")
                            nc.scalar.activation(P[:], sc[i][:], EXP, scale=0.125)
                            if kp % 2 == 0:
                                Up[i] = uacc([65, 512])
                            for j in range(2):
                                kt = 2 * kp + j
                                nc.tensor.matmul(
                                    Up[i][:],
                                    Vaug[kt][:, 2 * hp + i, :],
                                    P[:, 512 * j:512 * (j + 1)],
                                    start=(kt % 4 == 0), stop=(kt % 4 == 3))
                            if kp % 2 == 1:
                                if kp == 1:
                                    nc.vector.tensor_copy(Usb[i][:], Up[i][:])
                                else:
                                    nc.vector.tensor_add(Usb[i][:], Usb[i][:],
                                                         Up[i][:])
                    # normalize rows 0..63 of U by row 64, into out^T.
                    # Engine ops keep all operands on one partition range;
                    # cross-partition moves go through SBUF-SBUF DMA.
                    ot = persist.tile([128, 512], bf16, tag=f"oT{t}_{qc % 2}",
                                      name="oT")
                    outT[t] = ot
                    for i in range(2):
                        rrow = rpool.tile([1, 512], f32, tag="rrow", name="rrow")
                        nc.sync.dma_start(rrow[:], Usb[i][64:65, :])
                        rrec = rpool.tile([1, 512], f32, tag="rrec", name="rrec")
                        nc.vector.reciprocal_approx_fast(rrec[:], rrow[:])
                        rb = rpool.tile([64, 512], f32, tag="rb", name="rb")
                        nc.gpsimd.partition_broadcast(rb[:], rrec[:])
                        if i == 0:
                            nc.vector.tensor_mul(ot[0:64, :], Usb[i][0:64, :],
                                                 rb[:])
                        else:
                            stg = rpool.tile([64, 512], bf16, tag="stg",
                                             name="stg")
                            nc.vector.tensor_mul(stg[:], Usb[i][0:64, :], rb[:])
                            nc.sync.dma_start(ot[64:128, :], stg[:])

                # output projection for this q-chunk:
                # final[s, n] = sum_dv outT[dv, s] * woT[dv, n]
                for st in range(4):
                    for ncol in range(2):
                        acc = uacc([128, 512])
                        for t in range(NT):
                            nc.tensor.matmul(
                                acc[:],
                                outT[t][:, 128 * st:128 * (st + 1)],
                                wos[t][:, 512 * ncol:512 * (ncol + 1)],
                                start=(t == 0), stop=(t == NT - 1))
                        ob = obuf.tile([128, 512], f32, tag="ob", name="ob")
                        nc.vector.tensor_copy(ob[:], acc[:])
                        nc.sync.dma_start(
                            out[512 * qc + 128 * st:512 * qc + 128 * (st + 1),
                                512 * ncol:512 * (ncol + 1)],
                            ob[:])

    nc.compile()
    return nc


def kernel(query, key, value, w_q, w_k, w_v, w_o):
    import ml_dtypes
    from concourse.bass_utils import run_bass_kernel_spmd

    if "nc" not in _cached:
        _cached["nc"] = _build()
    nc = _cached["nc"]

    bf = ml_dtypes.bfloat16
    query = np.asarray(query, dtype=np.float32)
    key = np.asarray(key, dtype=np.float32)
    value = np.asarray(value, dtype=np.float32)
    w_q = np.asarray(w_q, dtype=np.float32)
    w_k = np.asarray(w_k, dtype=np.float32)
    w_v = np.asarray(w_v, dtype=np.float32)
    w_o = np.asarray(w_o, dtype=np.float32)

    def c(a):
        return np.ascontiguousarray(a).astype(bf)

    in_maps = []
    for core in range(N_CORES):
        b, g = core // G, core % G
        rows = slice(DV * g, DV * (g + 1))
        in_maps.append({
            "xqT": c(query[b].T),
            "xkT": c(key[b].T),
            "xvT": c(value[b].T),
            "wqT": c(w_q[rows, :].T),
            "wkT": c(w_k[rows, :].T),
            "wvT": c(w_v[rows, :].T),
            "woT": c(w_o[:, rows].T),
        })

    res = run_bass_kernel_spmd(nc, in_maps, list(range(N_CORES)))
    full = np.empty((B, S, D), np.float32)
    for b in range(B):
        full[b] = res.results[G * b]["out"] + res.results[G * b + 1]["out"]
    return full


# revision 11
# speedup vs baseline: 1.2905x; 1.0424x over previous
"""Multi-head attention (B=4, S=2048, D=1024, H=16) on 8 TRN2 NeuronCores.

Sharding: data-parallel over batch (4) x tensor-parallel over head halves (2).
Core c handles batch b = c//2 and heads [8g, 8g+8) where g = c%2.
Each core computes a partial [S, D] output-projection contribution; the host
sums the two head-group partials per batch.

All activations are passed to the device pre-transposed (feature dim on
partitions) so the kernel needs no on-device transposes:
  - projections contract over d (model dim) with host-transposed x^T,
  - scores are built transposed [k, q] so exp() output feeds the P @ V
    matmul directly as the moving operand,
  - P @ [V | 1] yields the softmax denominator as row 64 of U^T for free,
  - normalized U^T tiles are exactly the stationary layout w_o needs.

Matmul operands are bf16 (fp32 PSUM accumulation); fp32 moving operands
stream at half rate on TRN2, bf16 at full rate. Head pairs share the PE
array via row tiling (partitions 0-63 / 64-127) so the DK=64 score matmuls
run concurrently.

Emission order is tuned so the scalar engine (exp is the critical path,
~272us busy) starts early and never starves: the first head pair's
attention interleaves with K/V projection chunks, later q-chunks' Q
projections and the w_o output projections are injected between head
pairs of the running attention instead of clustering at chunk boundaries.

PSUM budget (8 banks): 3 x [128,1024] score tiles (6 banks) + one shared
2-slot pool (2 banks) for every [<=128,512] accumulator (projection accs,
P@V partials, w_o accs); attention U accumulates in SBUF via DVE adds of
4-k-tile PSUM partials so no PSUM slot is held for long.
"""

import numpy as np

B, S, D, H = 4, 2048, 1024, 16
DK = D // H          # 64
G = 2                # head groups (tensor-parallel degree per batch)
HL = H // G          # 8 local heads per core
DV = HL * DK         # 512 local value dim
N_CORES = 8

_cached = {}


def _build():
    import concourse.bass as bass
    import concourse.tile as tile
    from concourse import bacc, mybir

    f32 = mybir.dt.float32
    bf16 = mybir.dt.bfloat16
    EXP = mybir.ActivationFunctionType.Exp

    nc = bacc.Bacc("TRN2", target_bir_lowering=False, debug=False,
                   num_devices=N_CORES)

    xqT = nc.dram_tensor("xqT", [D, S], bf16, kind="ExternalInput").ap()
    xkT = nc.dram_tensor("xkT", [D, S], bf16, kind="ExternalInput").ap()
    xvT = nc.dram_tensor("xvT", [D, S], bf16, kind="ExternalInput").ap()
    wqT = nc.dram_tensor("wqT", [D, DV], bf16, kind="ExternalInput").ap()
    wkT = nc.dram_tensor("wkT", [D, DV], bf16, kind="ExternalInput").ap()
    wvT = nc.dram_tensor("wvT", [D, DV], bf16, kind="ExternalInput").ap()
    woT = nc.dram_tensor("woT", [DV, D], bf16, kind="ExternalInput").ap()
    out = nc.dram_tensor("out", [S, D], f32, kind="ExternalOutput").ap()

    ND = D // 128     # 8 d-tiles
    NS = S // 128     # 16 s-tiles (k-tiles)
    NQC = S // 512    # 4 q-chunks
    NT = DV // 128    # 4 dk/dv-tiles
    NHP = HL // 2     # 4 head pairs

    with tile.TileContext(nc) as tc:
        with (
            tc.tile_pool(name="persist", bufs=1) as persist,
            tc.tile_pool(name="stage", bufs=32) as stage,
            tc.tile_pool(name="wpool", bufs=8) as wpool,
            tc.tile_pool(name="spool", bufs=3, space=bass.MemorySpace.PSUM) as spool,
            tc.tile_pool(name="upool", bufs=2, space=bass.MemorySpace.PSUM) as upool,
            tc.tile_pool(name="ppool", bufs=4) as ppool,
            tc.tile_pool(name="rpool", bufs=3) as rpool,
            tc.tile_pool(name="obuf", bufs=3) as obuf,
        ):
            QT = {}    # [t][qc] -> [128, 512] tiles of Q^T (dk rows, q cols)
            KT = {}    # [t][c]  -> [128, 512]
            Vaug = {}  # [kt] -> [128, 8, 65]: per-head V columns + ones col
            outT = {}  # [qc][t] -> [128, 512] normalized attention out^T
            wks, wvs, wqs, wos = [], [], [], []
            st_ = {}   # per (qc, hp) attention state

            def uacc(shape):
                return upool.tile(shape, f32, tag="u", name="uacc")

            def emit_w_loads():
                for d in range(ND):
                    wt = wpool.tile([128, DV], bf16, tag="w", name="wk",
                                    bufs=24)
                    nc.sync.dma_start(wt[:], wkT[128 * d:128 * (d + 1), :])
                    wks.append(wt)
                for d in range(ND):
                    wt = wpool.tile([128, DV], bf16, tag="w", name="wv",
                                    bufs=24)
                    nc.sync.dma_start(wt[:], wvT[128 * d:128 * (d + 1), :])
                    wvs.append(wt)
                for d in range(ND):
                    wt = wpool.tile([128, DV], bf16, tag="w", name="wq",
                                    bufs=24)
                    nc.sync.dma_start(wt[:], wqT[128 * d:128 * (d + 1), :])
                    wqs.append(wt)
                for t in range(NT):
                    wo = wpool.tile([128, D], bf16, tag=f"wo{t}", name="wo",
                                    bufs=1)
                    nc.sync.dma_start(wo[:], woT[128 * t:128 * (t + 1), :])
                    wos.append(wo)

            def emit_kv_chunk(c):
                # K^T[dk, 512c:+512] for all t, and Vaug[4c..4c+4]
                xks = []
                for d in range(ND):
                    xt = stage.tile([128, 512], bf16, tag="act", name="xk")
                    nc.sync.dma_start(
                        xt[:], xkT[128 * d:128 * (d + 1), 512 * c:512 * (c + 1)])
                    xks.append(xt)
                xvs = []
                for d in range(ND):
                    xt = stage.tile([128, 512], bf16, tag="act", name="xv")
                    nc.sync.dma_start(
                        xt[:], xvT[128 * d:128 * (d + 1), 512 * c:512 * (c + 1)])
                    xvs.append(xt)
                for t in range(NT):
                    acc = uacc([128, 512])
                    for d in range(ND):
                        nc.tensor.matmul(
                            acc[:], wks[d][:, 128 * t:128 * (t + 1)], xks[d][:],
                            start=(d == 0), stop=(d == ND - 1))
                    dt_ = persist.tile([128, 512], bf16, tag=f"kT{t}_{c}",
                                       name="kT")
                    nc.vector.tensor_copy(dt_[:], acc[:])
                    KT.setdefault(t, {})[c] = dt_
                for ktl in range(4):
                    kt = 4 * c + ktl
                    acc = uacc([128, 512])
                    for d in range(ND):
                        nc.tensor.matmul(
                            acc[:], xvs[d][:, 128 * ktl:128 * (ktl + 1)],
                            wvs[d][:],
                            start=(d == 0), stop=(d == ND - 1))
                    va = persist.tile([128, HL, DK + 1], bf16, tag=f"vaug{kt}",
                                      name="vaug")
                    nc.vector.tensor_copy(
                        va[:, :, 0:DK],
                        acc[:].rearrange("p (h k) -> p h k", h=HL))
                    nc.vector.tensor_copy(
                        va[:, :, DK], nc.const_aps.tensor(1.0, (128, HL), bf16))
                    Vaug[kt] = va

            def emit_qproj(qc):
                xs = []
                for d in range(ND):
                    xt = stage.tile([128, 512], bf16, tag="act", name="xq")
                    nc.sync.dma_start(
                        xt[:], xqT[128 * d:128 * (d + 1), 512 * qc:512 * (qc + 1)])
                    xs.append(xt)
                for t in range(NT):
                    acc = uacc([128, 512])
                    for d in range(ND):
                        nc.tensor.matmul(
                            acc[:], wqs[d][:, 128 * t:128 * (t + 1)], xs[d][:],
                            start=(d == 0), stop=(d == ND - 1))
                    dt_ = persist.tile([128, 512], bf16, tag=f"qT{t}_{qc}",
                                       name="qT")
                    nc.vector.tensor_copy(dt_[:], acc[:])
                    QT.setdefault(t, {})[qc] = dt_

            def emit_attn(qc, hp, kps):
                # head pair (2hp, 2hp+1) = partition halves of tile hp: their
                # DK=64 score matmuls row-tile the PE array (rows 0-63/64-127).
                t = hp
                s = st_.setdefault((qc, hp), {"Usb": None, "Up": [None, None]})
                if s["Usb"] is None:
                    s["Usb"] = [rpool.tile([65, 512], f32, tag=f"usb{i}",
                                           name="usb") for i in range(2)]
                for kp in kps:
                    sc = [spool.tile([128, 1024], f32, tag="sc", name="sc")
                          for _ in range(2)]
                    for j in range(2):
                        kt = 2 * kp + j
                        for i in range(2):
                            po = 64 * i
                            nc.tensor.matmul(
                                sc[i][:, 512 * j:512 * (j + 1)],
                                KT[t][kt // 4][po:po + 64,
                                               128 * (kt % 4):128 * (kt % 4 + 1)],
                                QT[t][qc][po:po + 64, :],
                                start=True, stop=True)
                    for i in range(2):
                        P = ppool.tile([128, 1024], bf16, tag="p", name="p")
                        nc.scalar.activation(P[:], sc[i][:], EXP, scale=0.125)
                        if kp % 2 == 0:
                            s["Up"][i] = uacc([65, 512])
                        for j in range(2):
                            kt = 2 * kp + j
                            nc.tensor.matmul(
                                s["Up"][i][:],
                                Vaug[kt][:, 2 * hp + i, :],
                                P[:, 512 * j:512 * (j + 1)],
                                start=(kt % 4 == 0), stop=(kt % 4 == 3))
                        if kp % 2 == 1:
                            if kp == 1:
                                nc.vector.tensor_copy(s["Usb"][i][:],
                                                      s["Up"][i][:])
                            else:
                                nc.vector.tensor_add(s["Usb"][i][:],
                                                     s["Usb"][i][:],
                                                     s["Up"][i][:])

            def emit_normalize(qc, hp):
                # rows 0..63 of U divided by row 64 (the ones-column sum),
                # written into out^T. Engine ops keep operands on one
                # partition range; cross-partition moves via SBUF-SBUF DMA.
                t = hp
                Usb = st_[(qc, hp)]["Usb"]
                ot = persist.tile([128, 512], bf16, tag=f"oT{t}_{qc % 2}",
                                  name="oT")
                outT.setdefault(qc, {})[t] = ot
                for i in range(2):
                    rrow = rpool.tile([1, 512], f32, tag="rrow", name="rrow")
                    nc.sync.dma_start(rrow[:], Usb[i][64:65, :])
                    rrec = rpool.tile([1, 512], f32, tag="rrec", name="rrec")
                    nc.vector.reciprocal_approx_fast(rrec[:], rrow[:])
                    rb = rpool.tile([64, 512], f32, tag="rb", name="rb")
                    nc.gpsimd.partition_broadcast(rb[:], rrec[:])
                    if i == 0:
                        nc.vector.tensor_mul(ot[0:64, :], Usb[i][0:64, :],
                                             rb[:])
                    else:
                        stg = rpool.tile([64, 512], bf16, tag="stg",
                                         name="stg")
                        nc.vector.tensor_mul(stg[:], Usb[i][0:64, :], rb[:])
                        nc.sync.dma_start(ot[64:128, :], stg[:])

            def emit_wo(qc):
                # final[s, n] = sum_dv outT[dv, s] * woT[dv, n]
                for st in range(4):
                    for ncol in range(2):
                        acc = uacc([128, 512])
                        for t in range(NT):
                            nc.tensor.matmul(
                                acc[:],
                                outT[qc][t][:, 128 * st:128 * (st + 1)],
                                wos[t][:, 512 * ncol:512 * (ncol + 1)],
                                start=(t == 0), stop=(t == NT - 1))
                        ob = obuf.tile([128, 512], f32, tag="ob", name="ob")
                        nc.vector.tensor_copy(ob[:], acc[:])
                        nc.sync.dma_start(
                            out[512 * qc + 128 * st:512 * qc + 128 * (st + 1),
                                512 * ncol:512 * (ncol + 1)],
                            ob[:])

            # ---- emission schedule ----
            emit_w_loads()
            emit_kv_chunk(0)
            emit_qproj(0)
            # first head pair interleaves with remaining K/V chunks: scores
            # for k-tiles [4c, 4c+4) only need K/V chunk c.
            emit_attn(0, 0, range(0, 2))
            emit_kv_chunk(1)
            emit_attn(0, 0, range(2, 4))
            emit_kv_chunk(2)
            emit_attn(0, 0, range(4, 6))
            emit_kv_chunk(3)
            emit_attn(0, 0, range(6, 8))
            emit_normalize(0, 0)
            for hp in range(1, NHP):
                emit_attn(0, hp, range(8))
                emit_normalize(0, hp)
            emit_qproj(1)
            for qc in range(1, NQC):
                for hp in range(NHP):
                    emit_attn(qc, hp, range(8))
                    emit_normalize(qc, hp)
                    if hp == 0 and qc < NQC - 1:
                        emit_qproj(qc + 1)
                    if hp == 1:
                        emit_wo(qc - 1)
            emit_wo(NQC - 1)

    nc.compile()
    return nc


def kernel(query, key, value, w_q, w_k, w_v, w_o):
    import ml_dtypes
    from concourse.bass_utils import run_bass_kernel_spmd

    if "nc" not in _cached:
        _cached["nc"] = _build()
    nc = _cached["nc"]

    bf = ml_dtypes.bfloat16
    query = np.asarray(query, dtype=np.float32)
    key = np.asarray(key, dtype=np.float32)
    value = np.asarray(value, dtype=np.float32)
    w_q = np.asarray(w_q, dtype=np.float32)
    w_k = np.asarray(w_k, dtype=np.float32)
    w_v = np.asarray(w_v, dtype=np.float32)
    w_o = np.asarray(w_o, dtype=np.float32)

    def c(a):
        return np.ascontiguousarray(a).astype(bf)

    in_maps = []
    for core in range(N_CORES):
        b, g = core // G, core % G
        rows = slice(DV * g, DV * (g + 1))
        in_maps.append({
            "xqT": c(query[b].T),
            "xkT": c(key[b].T),
            "xvT": c(value[b].T),
            "wqT": c(w_q[rows, :].T),
            "wkT": c(w_k[rows, :].T),
            "wvT": c(w_v[rows, :].T),
            "woT": c(w_o[:, rows].T),
        })

    res = run_bass_kernel_spmd(nc, in_maps, list(range(N_CORES)))
    full = np.empty((B, S, D), np.float32)
    for b in range(B):
        full[b] = res.results[G * b]["out"] + res.results[G * b + 1]["out"]
    return full


# revision 12
# speedup vs baseline: 1.3044x; 1.0108x over previous
"""Multi-head attention (B=4, S=2048, D=1024, H=16) on 8 TRN2 NeuronCores.

Sharding: data-parallel over batch (4) x tensor-parallel over head halves (2).
Core c handles batch b = c//2 and heads [8g, 8g+8) where g = c%2.
Each core computes a partial [S, D] output-projection contribution; the host
sums the two head-group partials per batch.

All activations are passed to the device pre-transposed (feature dim on
partitions) so the kernel needs no on-device transposes:
  - projections contract over d (model dim) with host-transposed x^T,
  - scores are built transposed [k, q] so exp() output feeds the P @ V
    matmul directly as the moving operand,
  - P @ [V | 1] yields the softmax denominator as row 64 of U^T for free,
  - normalized U^T tiles are exactly the stationary layout w_o needs.

Matmul operands are bf16 (fp32 PSUM accumulation); fp32 moving operands
stream at half rate on TRN2, bf16 at full rate. Head pairs share the PE
array via row tiling (partitions 0-63 / 64-127) so the DK=64 score matmuls
run concurrently.

The scalar engine's exp() stream (~272us busy) is the critical path, and
its input buffer is only 3 PSUM score tiles (~3.4us of lookahead), so all
non-attention PE work (K/V/Q projection accumulations, w_o output
projection) is chopped into single-accumulator "pieces" and injected at
most two at a time between attention turns. Attention itself runs
kp-major: for each k-range, all four head pairs take a turn, giving 16
uniform injection slots per q-chunk.

PSUM budget (8 banks): 3 x [128,1024] score tiles (6 banks) + one shared
2-slot pool (2 banks) for every [<=128,512] accumulator; attention U
accumulates in SBUF via DVE adds of 4-k-tile PSUM partials so no PSUM slot
is held for long.
"""

import numpy as np

B, S, D, H = 4, 2048, 1024, 16
DK = D // H          # 64
G = 2                # head groups (tensor-parallel degree per batch)
HL = H // G          # 8 local heads per core
DV = HL * DK         # 512 local value dim
N_CORES = 8

_cached = {}


def _build():
    import concourse.bass as bass
    import concourse.tile as tile
    from concourse import bacc, mybir

    f32 = mybir.dt.float32
    bf16 = mybir.dt.bfloat16
    EXP = mybir.ActivationFunctionType.Exp

    nc = bacc.Bacc("TRN2", target_bir_lowering=False, debug=False,
                   num_devices=N_CORES)

    xqT = nc.dram_tensor("xqT", [D, S], bf16, kind="ExternalInput").ap()
    xkT = nc.dram_tensor("xkT", [D, S], bf16, kind="ExternalInput").ap()
    xvT = nc.dram_tensor("xvT", [D, S], bf16, kind="ExternalInput").ap()
    wqT = nc.dram_tensor("wqT", [D, DV], bf16, kind="ExternalInput").ap()
    wkT = nc.dram_tensor("wkT", [D, DV], bf16, kind="ExternalInput").ap()
    wvT = nc.dram_tensor("wvT", [D, DV], bf16, kind="ExternalInput").ap()
    woT = nc.dram_tensor("woT", [DV, D], bf16, kind="ExternalInput").ap()
    out = nc.dram_tensor("out", [S, D], f32, kind="ExternalOutput").ap()

    ND = D // 128     # 8 d-tiles
    NS = S // 128     # 16 s-tiles (k-tiles)
    NQC = S // 512    # 4 q-chunks
    NT = DV // 128    # 4 dk/dv-tiles
    NHP = HL // 2     # 4 head pairs

    with tile.TileContext(nc) as tc:
        with (
            tc.tile_pool(name="persist", bufs=1) as persist,
            tc.tile_pool(name="stage", bufs=32) as stage,
            tc.tile_pool(name="wpool", bufs=8) as wpool,
            tc.tile_pool(name="spool", bufs=3, space=bass.MemorySpace.PSUM) as spool,
            tc.tile_pool(name="upool", bufs=2, space=bass.MemorySpace.PSUM) as upool,
            tc.tile_pool(name="ppool", bufs=4) as ppool,
            tc.tile_pool(name="rpool", bufs=3) as rpool,
            tc.tile_pool(name="obuf", bufs=3) as obuf,
        ):
            QT = {}    # [t][qc] -> [128, 512] tiles of Q^T (dk rows, q cols)
            KT = {}    # [t][c]  -> [128, 512]
            Vaug = {}  # [kt] -> [128, 8, 65]: per-head V columns + ones col
            outT = {}  # [qc][t] -> [128, 512] normalized attention out^T
            wks, wvs, wqs, wos = [], [], [], []
            st_ = {}   # per (qc, hp) attention state
            xq_stage = {}

            def uacc(shape):
                return upool.tile(shape, f32, tag="u", name="uacc")

            def emit_w_loads():
                for lst, name, src in ((wks, "wk", wkT), (wvs, "wv", wvT),
                                       (wqs, "wq", wqT)):
                    for d in range(ND):
                        wt = wpool.tile([128, DV], bf16, tag="w", name=name,
                                        bufs=24)
                        nc.sync.dma_start(wt[:], src[128 * d:128 * (d + 1), :])
                        lst.append(wt)
                for t in range(NT):
                    wo = wpool.tile([128, D], bf16, tag=f"wo{t}", name="wo",
                                    bufs=1)
                    nc.sync.dma_start(wo[:], woT[128 * t:128 * (t + 1), :])
                    wos.append(wo)

            def emit_kv_dmas(c):
                xks, xvs = [], []
                for d in range(ND):
                    xt = stage.tile([128, 512], bf16, tag="act", name="xk")
                    nc.sync.dma_start(
                        xt[:], xkT[128 * d:128 * (d + 1), 512 * c:512 * (c + 1)])
                    xks.append(xt)
                for d in range(ND):
                    xt = stage.tile([128, 512], bf16, tag="act", name="xv")
                    nc.sync.dma_start(
                        xt[:], xvT[128 * d:128 * (d + 1), 512 * c:512 * (c + 1)])
                    xvs.append(xt)
                return xks, xvs

            def piece_kproj(xks, c, t):
                def go():
                    acc = uacc([128, 512])
                    for d in range(ND):
                        nc.tensor.matmul(
                            acc[:], wks[d][:, 128 * t:128 * (t + 1)], xks[d][:],
                            start=(d == 0), stop=(d == ND - 1))
                    dt_ = persist.tile([128, 512], bf16, tag=f"kT{t}_{c}",
                                       name="kT")
                    nc.vector.tensor_copy(dt_[:], acc[:])
                    KT.setdefault(t, {})[c] = dt_
                return go

            def piece_vproj(xvs, c, ktl):
                def go():
                    kt = 4 * c + ktl
                    acc = uacc([128, 512])
                    for d in range(ND):
                        nc.tensor.matmul(
                            acc[:], xvs[d][:, 128 * ktl:128 * (ktl + 1)],
                            wvs[d][:],
                            start=(d == 0), stop=(d == ND - 1))
                    va = persist.tile([128, HL, DK + 1], bf16, tag=f"vaug{kt}",
                                      name="vaug")
                    nc.vector.tensor_copy(
                        va[:, :, 0:DK],
                        acc[:].rearrange("p (h k) -> p h k", h=HL))
                    nc.vector.tensor_copy(
                        va[:, :, DK], nc.const_aps.tensor(1.0, (128, HL), bf16))
                    Vaug[kt] = va
                return go

            def emit_xq_dmas(qc):
                xs = []
                for d in range(ND):
                    xt = stage.tile([128, 512], bf16, tag="act", name="xq")
                    nc.sync.dma_start(
                        xt[:], xqT[128 * d:128 * (d + 1), 512 * qc:512 * (qc + 1)])
                    xs.append(xt)
                xq_stage[qc] = xs

            def piece_qproj(qc, t):
                def go():
                    xs = xq_stage[qc]
                    acc = uacc([128, 512])
                    for d in range(ND):
                        nc.tensor.matmul(
                            acc[:], wqs[d][:, 128 * t:128 * (t + 1)], xs[d][:],
                            start=(d == 0), stop=(d == ND - 1))
                    dt_ = persist.tile([128, 512], bf16, tag=f"qT{t}_{qc}",
                                       name="qT")
                    nc.vector.tensor_copy(dt_[:], acc[:])
                    QT.setdefault(t, {})[qc] = dt_
                return go

            def piece_wo(qc, st, ncol):
                # final[s, n] = sum_dv outT[dv, s] * woT[dv, n]
                def go():
                    acc = uacc([128, 512])
                    for t in range(NT):
                        nc.tensor.matmul(
                            acc[:],
                            outT[qc][t][:, 128 * st:128 * (st + 1)],
                            wos[t][:, 512 * ncol:512 * (ncol + 1)],
                            start=(t == 0), stop=(t == NT - 1))
                    ob = obuf.tile([128, 512], f32, tag="ob", name="ob")
                    nc.vector.tensor_copy(ob[:], acc[:])
                    nc.sync.dma_start(
                        out[512 * qc + 128 * st:512 * qc + 128 * (st + 1),
                            512 * ncol:512 * (ncol + 1)],
                        ob[:])
                return go

            def emit_attn_turn(qc, hp, c):
                # head pair (2hp, 2hp+1) = partition halves of tile hp: their
                # DK=64 score matmuls row-tile the PE array (rows 0-63/64-127).
                # One turn covers k-tiles [4c, 4c+4) = one PSUM partial group,
                # folded into the SBUF accumulator Usb.
                t = hp
                s = st_.setdefault((qc, hp), {})
                if c == 0:
                    s["Usb"] = [rpool.tile([65, 512], f32, tag=f"usb{hp}_{i}",
                                           name="usb", bufs=2)
                                for i in range(2)]
                Up = [None, None]
                for kp in (2 * c, 2 * c + 1):
                    sc = [spool.tile([128, 1024], f32, tag="sc", name="sc")
                          for _ in range(2)]
                    for j in range(2):
                        kt = 2 * kp + j
                        for i in range(2):
                            po = 64 * i
                            nc.tensor.matmul(
                                sc[i][:, 512 * j:512 * (j + 1)],
                                KT[t][kt // 4][po:po + 64,
                                               128 * (kt % 4):128 * (kt % 4 + 1)],
                                QT[t][qc][po:po + 64, :],
                                start=True, stop=True)
                    for i in range(2):
                        P = ppool.tile([128, 1024], bf16, tag="p", name="p")
                        nc.scalar.activation(P[:], sc[i][:], EXP, scale=0.125)
                        if kp % 2 == 0:
                            Up[i] = uacc([65, 512])
                        for j in range(2):
                            kt = 2 * kp + j
                            nc.tensor.matmul(
                                Up[i][:],
                                Vaug[kt][:, 2 * hp + i, :],
                                P[:, 512 * j:512 * (j + 1)],
                                start=(kt % 4 == 0), stop=(kt % 4 == 3))
                        if kp % 2 == 1:
                            if c == 0:
                                nc.vector.tensor_copy(s["Usb"][i][:], Up[i][:])
                            else:
                                nc.vector.tensor_add(s["Usb"][i][:],
                                                     s["Usb"][i][:], Up[i][:])

            def emit_normalize(qc, hp):
                # rows 0..63 of U divided by row 64 (the ones-column sum),
                # written into out^T. Engine ops keep operands on one
                # partition range; cross-partition moves via SBUF-SBUF DMA.
                t = hp
                Usb = st_[(qc, hp)]["Usb"]
                ot = persist.tile([128, 512], bf16, tag=f"oT{t}_{qc % 2}",
                                  name="oT")
                outT.setdefault(qc, {})[t] = ot
                for i in range(2):
                    rrow = rpool.tile([1, 512], f32, tag="rrow", name="rrow")
                    nc.sync.dma_start(rrow[:], Usb[i][64:65, :])
                    rrec = rpool.tile([1, 512], f32, tag="rrec", name="rrec")
                    nc.vector.reciprocal_approx_fast(rrec[:], rrow[:])
                    rb = rpool.tile([64, 512], f32, tag="rb", name="rb")
                    nc.gpsimd.partition_broadcast(rb[:], rrec[:])
                    if i == 0:
                        nc.vector.tensor_mul(ot[0:64, :], Usb[i][0:64, :],
                                             rb[:])
                    else:
                        stg = rpool.tile([64, 512], bf16, tag="stg",
                                         name="stg")
                        nc.vector.tensor_mul(stg[:], Usb[i][0:64, :], rb[:])
                        nc.sync.dma_start(ot[64:128, :], stg[:])

            # ---- emission schedule ----
            emit_w_loads()
            xks0, xvs0 = emit_kv_dmas(0)
            emit_xq_dmas(0)
            for ktl in range(4):
                piece_vproj(xvs0, 0, ktl)()
            for t in range(NT):
                piece_kproj(xks0, 0, t)()
            for t in range(NT):
                piece_qproj(0, t)()

            for qc in range(NQC):
                # build this q-chunk's injection queue (V before K inside a
                # chunk so the chunk is fully usable as soon as possible)
                pieces = []
                if qc == 0:
                    for c2 in range(1, 4):
                        xks, xvs = emit_kv_dmas(c2)
                        for ktl in range(4):
                            pieces.append(piece_vproj(xvs, c2, ktl))
                        for t in range(NT):
                            pieces.append(piece_kproj(xks, c2, t))
                if qc < NQC - 1:
                    emit_xq_dmas(qc + 1)
                    for t in range(NT):
                        pieces.append(piece_qproj(qc + 1, t))
                if qc > 0:
                    for st2 in range(4):
                        for ncol in range(2):
                            pieces.append(piece_wo(qc - 1, st2, ncol))
                # 16 turns, up to 2 pieces after each
                per_slot = -(-len(pieces) // 16)
                pi = 0
                for c in range(4):
                    for hp in range(NHP):
                        emit_attn_turn(qc, hp, c)
                        for _ in range(per_slot):
                            if pi < len(pieces):
                                pieces[pi]()
                                pi += 1
                        if c == 3:
                            emit_normalize(qc, hp)
                assert pi == len(pieces)
            for st2 in range(4):
                for ncol in range(2):
                    piece_wo(NQC - 1, st2, ncol)()

    nc.compile()
    return nc


def kernel(query, key, value, w_q, w_k, w_v, w_o):
    import ml_dtypes
    from concourse.bass_utils import run_bass_kernel_spmd

    if "nc" not in _cached:
        _cached["nc"] = _build()
    nc = _cached["nc"]

    bf = ml_dtypes.bfloat16
    query = np.asarray(query, dtype=np.float32)
    key = np.asarray(key, dtype=np.float32)
    value = np.asarray(value, dtype=np.float32)
    w_q = np.asarray(w_q, dtype=np.float32)
    w_k = np.asarray(w_k, dtype=np.float32)
    w_v = np.asarray(w_v, dtype=np.float32)
    w_o = np.asarray(w_o, dtype=np.float32)

    def c(a):
        return np.ascontiguousarray(a).astype(bf)

    in_maps = []
    for core in range(N_CORES):
        b, g = core // G, core % G
        rows = slice(DV * g, DV * (g + 1))
        in_maps.append({
            "xqT": c(query[b].T),
            "xkT": c(key[b].T),
            "xvT": c(value[b].T),
            "wqT": c(w_q[rows, :].T),
            "wkT": c(w_k[rows, :].T),
            "wvT": c(w_v[rows, :].T),
            "woT": c(w_o[:, rows].T),
        })

    res = run_bass_kernel_spmd(nc, in_maps, list(range(N_CORES)))
    full = np.empty((B, S, D), np.float32)
    for b in range(B):
        full[b] = res.results[G * b]["out"] + res.results[G * b + 1]["out"]
    return full
